# revision 65
# baseline (speedup 1.0000x reference)
"""AttentionMixer Trainium2 kernel — 8-core data-parallel (batch sharded).

Host folds the projection chain (W_lq, Wq, Wk on 7 gathered rows per batch)
into per-batch query vectors qW [B,14,H] (bf16) and supplies emb in TWO
pre-transposed bf16 layouts so the device never transposes emb:
  embs [S, NB, H]  (s on partitions)  — weighted-sum stationary
  embh [H, NB, S]  (H on partitions)  — scores moving operand

Device per 64-batch group (stage-skewed software pipeline: s1 of group g+1
is hoisted a cycle early, s4 lags one group and s5 two groups, so each
in-order engine stream interleaves phases of three groups):
  s1: scores = qW @ embh (PE bf16, 4 col-tiled quadrant matmuls/supergroup),
      then sigma via 0.5*tanh(x/2)+0.5 (ACT) — tanh/exp/copy/square share one
      ACT table so the main chain is table-resident
  s2: E = exp(sigma) with Z accumulation (ACT, same table)
  s3: alpha1^2 = (E*rz^2)*E via DVE scalar_tensor_tensor; alpha1^4 via one
      GpSimd tensor_tensor square; LP pool over levels via 0/1 selection
      matmuls (PE bf16)
  s4: p = t^(1/4) (ACT sqrt x2, batched k to one table load); e0 = exp(p);
      masked softmax over S (DVE), alpha transposed via PE f32r
  s5: weighted sum via per-batch bf16 FWL stationary matmuls (PE, N=2)

Layouts (per 64-batch group, batch bl = 8*u + 2*q + par, u = 4*k + v):
  scores/sig/E tiles [128, 400] per supergroup u: row 32*q + 14*par + (7h+l),
    col 200*par + s.
  t/e tiles [128, 400] per half-group k: row 32*v + 8*par + 2*q + h.
"""

import numpy as np

N_CORES = 8
B, S, H = 2048, 200, 128
L, NH, D = 7, 2, 64
NB = B // N_CORES          # 256 batches per core
GRP = 64                   # batches per group
NGRP = NB // GRP           # 4 groups per core
SG = 8                     # batches per supergroup
NSG = GRP // SG            # 8 supergroups per group
SA, SB_ = 128, 72          # s-tile split 200 = 128 + 72

_CACHE = {}


def _build_nc():
    import concourse.bacc as bacc
    import concourse.mybir as mybir
    import concourse.tile as tile

    fp32 = mybir.dt.float32
    f32r = mybir.dt.float32r
    bf16 = mybir.dt.bfloat16
    ACT = mybir.ActivationFunctionType
    ALU = mybir.AluOpType
    AX = mybir.AxisListType

    nc = bacc.Bacc(None, target_bir_lowering=False, debug=False)

    embs = nc.declare_dram_parameter("embs", [S, NB, H], bf16, isOutput=False)
    embh = nc.declare_dram_parameter("embh", [H, NB, S], bf16, isOutput=False)
    qwt = nc.declare_dram_parameter("qwt", [NGRP, H, (GRP // 2) * 32], bf16, isOutput=False)
    qbp = nc.declare_dram_parameter("qbp", [NGRP, 128, NSG], fp32, isOutput=False)
    msk = nc.declare_dram_parameter("msk", [NGRP, 2, 128, 400], bf16, isOutput=False)
    sel = nc.declare_dram_parameter("sel", [128, 32], bf16, isOutput=False)
    idn = nc.declare_dram_parameter("idn", [128, 128], fp32, isOutput=False)
    out = nc.declare_dram_parameter("out", [NB, H], fp32, isOutput=True)

    def r(ap):
        return ap.bitcast(f32r)

    with tile.TileContext(nc) as tc:
        with (
            tc.tile_pool(name="const", bufs=1) as constp,
            tc.tile_pool(name="io", bufs=3) as iop,
            tc.tile_pool(name="embt", bufs=2) as embtp,
            tc.tile_pool(name="embs", bufs=11) as embsp,
            tc.tile_pool(name="sig", bufs=17) as sigp,
            tc.tile_pool(name="ee", bufs=8) as eep,
            tc.tile_pool(name="work", bufs=2) as workp,
            tc.tile_pool(name="e4", bufs=8) as e4p,
            tc.tile_pool(name="ens", bufs=6) as ensp,
            tc.tile_pool(name="small", bufs=2) as smallp,
            tc.tile_pool(name="psB", bufs=2, space="PSUM") as psB,
            tc.tile_pool(name="psC", bufs=1, space="PSUM") as psC,
            tc.tile_pool(name="psD", bufs=2, space="PSUM") as psD,
            tc.tile_pool(name="psE", bufs=1, space="PSUM") as psE,
        ):
            selT = constp.tile([128, 32], bf16, tag="sel")
            nc.sync.dma_start(out=selT[:, :], in_=sel[:, :])
            idT = constp.tile([128, 128], fp32, tag="idn")
            nc.sync.dma_start(out=r(idT[:, :]), in_=r(idn[:, :]))
            outT = constp.tile([128, NB], fp32, tag="outT")
            halfT = constp.tile([128, 1], fp32, tag="half")
            nc.gpsimd.memset(halfT[:, :], 0.5)

            st = {}

            def s1_scores_tanh(g):
                b0 = g * GRP
                d = st[g] = {}
                qwtT = iop.tile([H, (GRP // 2) * 32], bf16, tag="qwt")
                nc.sync.dma_start(out=qwtT[:, :], in_=qwt[g, :, :])
                mskT = [iop.tile([128, 400], bf16, tag=f"msk{k}", name=f"mskT{k}g{g}")
                        for k in range(2)]
                for k in range(2):
                    nc.sync.dma_start(out=mskT[k][:, :], in_=msk[g, k, :, :])
                qbT = iop.tile([128, NSG], fp32, tag="qb")
                nc.sync.dma_start(out=qbT[:, :], in_=qbp[g, :, :])
                embT = embtp.tile([128, GRP * S], bf16, tag="embT")
                qgrp = GRP // 8
                for qq in range(8):
                    nc.sync.dma_start(
                        out=embT[:, qq * qgrp * S:(qq + 1) * qgrp * S],
                        in_=embh[:, b0 + qq * qgrp:b0 + (qq + 1) * qgrp, :])
                embAs, embBs = [], []
                for w in range(NSG // 2):
                    bu = b0 + w * 2 * SG
                    embA = embsp.tile([SA, 2 * SG, H], bf16, tag="embA")
                    nc.sync.dma_start(out=embA[:, :, :], in_=embs[0:SA, bu:bu + 2 * SG, :])
                    embB = embsp.tile([SB_, 2 * SG, H], bf16, tag="embB")
                    nc.sync.dma_start(out=embB[:, :, :], in_=embs[SA:S, bu:bu + 2 * SG, :])
                    embAs.append(embA)
                    embBs.append(embB)
                d["mskT"], d["embAs"], d["embBs"] = mskT, embAs, embBs

                sigTs = []
                for u in range(NSG):
                    scP = psB.tile([128, 400], fp32, tag="scores")
                    for q in range(SG // 2):
                        c0 = 32 * (u * (SG // 2) + q)
                        cb = (8 * u + 2 * q) * S
                        nc.tensor.matmul(
                            scP[32 * q:32 * q + 32, :],
                            qwtT[:, c0:c0 + 32],
                            embT[:, cb:cb + 2 * S],
                            start=True, stop=True, tile_position=(0, 32 * q))
                    sigT = sigp.tile([128, 400], bf16, tag="sig")
                    nc.scalar.activation(sigT[:, :], scP[:, :], ACT.Tanh,
                                         scale=0.5, bias=qbT[:, u:u + 1])
                    sigTs.append(sigT)
                d["sigTs"] = sigTs

            def s2_exp(g):
                d = st[g]
                zT = smallp.tile([128, 2 * NSG], fp32, tag="z")
                eTs = []
                for u in range(NSG):
                    eT = eep.tile([128, 400], fp32, tag="E")
                    for par in range(2):
                        nc.scalar.activation(
                            eT[:, 200 * par:200 * par + 200],
                            d["sigTs"][u][:, 200 * par:200 * par + 200],
                            ACT.Exp, scale=0.5, bias=halfT[:, 0:1],
                            accum_out=zT[:, 2 * u + par:2 * u + par + 1])
                    eTs.append(eT)
                d["eTs"], d["zT"] = eTs, zT

            def s3_e4_lp(g):
                d = st[g]
                rzT = smallp.tile([128, 2 * NSG], fp32, tag="rz")
                nc.vector.reciprocal(rzT[:, :], d["zT"][:, :])
                rz2T = smallp.tile([128, 2 * NSG], fp32, tag="rz2")
                nc.vector.tensor_tensor(rz2T[:, :], rzT[:, :], rzT[:, :], ALU.mult)
                e4Ts = []
                for u in range(NSG):
                    # C1 = (E*rz^2)*E = (E/Z)^2 ; e4 = C1^2 = (E/Z)^4
                    cT = workp.tile([128, 400], fp32, tag="c1")
                    for par in range(2):
                        sl = slice(200 * par, 200 * par + 200)
                        nc.vector.scalar_tensor_tensor(
                            cT[:, sl], d["eTs"][u][:, sl],
                            rz2T[:, 2 * u + par:2 * u + par + 1],
                            d["eTs"][u][:, sl], ALU.mult, ALU.mult)
                    e4T = e4p.tile([128, 400], bf16, tag="e4")
                    nc.gpsimd.tensor_tensor(e4T[:, :], cT[:, :], cT[:, :], ALU.mult)
                    e4Ts.append(e4T)
                tP = [psC.tile([128, 400], fp32, tag=f"t{_k}", name=f"tP{_k}g{g}")
                      for _k in range(2)]
                for u in range(NSG):
                    k, v = u // 4, u % 4
                    nc.tensor.matmul(tP[k][32 * v:32 * v + 32, :], selT[:, :],
                                     e4Ts[u][:, :], start=True, stop=True,
                                     tile_position=(0, 32 * v))
                d["tP"] = tP

            def s4_softmax2(g):
                d = st[g]
                enSs = []
                # both k's sqrts first, then both exps: one sqrt-table load and
                # one exp-table load per group instead of two of each
                tsqs = []
                for k in range(2):
                    tsq = workp.tile([128, 400], fp32, tag="tsq")
                    nc.scalar.activation(tsq[:, :], d["tP"][k][:, :], ACT.Sqrt)
                    nc.scalar.activation(tsq[:, :], tsq[:, :], ACT.Sqrt)
                    tsqs.append(tsq)
                e0Ts = []
                for k in range(2):
                    e0T = workp.tile([128, 400], fp32, tag="e0")
                    nc.scalar.activation(e0T[:, :], tsqs[k][:, :], ACT.Exp)
                    e0Ts.append(e0T)
                for k in range(2):
                    e0T = e0Ts[k]
                    enT = workp.tile([128, 400], fp32, tag="en")
                    nc.vector.tensor_tensor(r(enT[:, :]), e0T[:, :], d["mskT"][k][:, :],
                                            ALU.mult)
                    d2T = smallp.tile([128, 2], fp32, tag="d2")
                    nc.vector.tensor_reduce(d2T[:, 0:1], enT[:, 0:200], AX.X, ALU.add)
                    nc.vector.tensor_reduce(d2T[:, 1:2], enT[:, 200:400], AX.X, ALU.add)
                    drT = smallp.tile([128, 2], fp32, tag="dr")
                    nc.vector.tensor_scalar_add(d2T[:, :], d2T[:, :], 1e-30)
                    nc.vector.reciprocal(drT[:, :], d2T[:, :])
                    for par in range(2):
                        nc.vector.tensor_scalar_mul(
                            r(enT[:, 200 * par:200 * par + 200]),
                            enT[:, 200 * par:200 * par + 200], drT[:, par:par + 1])

                    enP = psE.tile([128, 512], fp32, tag="enat")
                    nc.tensor.transpose(r(enP[:, 0:128]), r(enT[:, 0:128]), r(idT[:, :]))
                    nc.tensor.transpose(r(enP[0:SB_, 128:256]), r(enT[:, 128:200]), r(idT[:, :]))
                    nc.tensor.transpose(r(enP[:, 256:384]), r(enT[:, 200:328]), r(idT[:, :]))
                    nc.tensor.transpose(r(enP[0:SB_, 384:512]), r(enT[:, 328:400]), r(idT[:, :]))
                    enS = ensp.tile([128, 512], bf16, tag="enS")
                    nc.vector.tensor_copy(enS[:, 0:128], enP[:, 0:128])
                    nc.vector.tensor_copy(enS[0:SB_, 128:256], enP[0:SB_, 128:256])
                    nc.vector.tensor_copy(enS[:, 256:384], enP[:, 256:384])
                    nc.vector.tensor_copy(enS[0:SB_, 384:512], enP[0:SB_, 384:512])
                    enSs.append(enS)
                d["enSs"] = enSs

            def s5_weighted(g):
                d = st[g]
                embAs, embBs, enSs = d["embAs"], d["embBs"], d["enSs"]
                oaP = psD.tile([128, 2 * GRP], fp32, tag="oacc")
                for bl in range(GRP):
                    u, r8 = bl // SG, bl % SG
                    k, v = u // 4, u % 4
                    q, par = r8 // 2, r8 % 2
                    r0 = 32 * v + 8 * par + 2 * q
                    cA = 0 if par == 0 else 256
                    cB = 128 if par == 0 else 384
                    nc.tensor.matmul(
                        oaP[:, 2 * bl:2 * bl + 2],
                        embAs[u // 2][:, (u % 2) * SG + r8, :],
                        enSs[k][0:SA, cA + r0:cA + r0 + 2],
                        start=True, stop=False)
                    nc.tensor.matmul(
                        oaP[:, 2 * bl:2 * bl + 2],
                        embBs[u // 2][:, (u % 2) * SG + r8, :],
                        enSs[k][0:SB_, cB + r0:cB + r0 + 2],
                        start=False, stop=True)
                oa3 = oaP[:, :].rearrange("p (b two) -> p b two", two=2)
                nc.vector.tensor_copy(r(outT[0:64, g * GRP:(g + 1) * GRP]), oa3[0:64, :, 0])
                nc.vector.tensor_copy(r(outT[64:128, g * GRP:(g + 1) * GRP]), oa3[64:128, :, 1])
                del st[g]

            # stage-skewed emission: each engine's in-order stream interleaves
            # group g's front phases with group g-1's back phases, so no engine
            # head-of-line blocks on the e4 chain.
            s1_scores_tanh(0)
            for g in range(NGRP):
                if g + 1 < NGRP:
                    s1_scores_tanh(g + 1)
                s2_exp(g)
                s3_e4_lp(g)
                if g >= 1:
                    s4_softmax2(g - 1)
                if g >= 2:
                    s5_weighted(g - 2)
            s4_softmax2(NGRP - 1)
            s5_weighted(NGRP - 2)
            s5_weighted(NGRP - 1)

            # final transpose outT [i, b] -> out [b, i]
            for kk in range(NB // 128):
                ofP = psE.tile([128, 128], fp32, tag="enat")
                nc.tensor.transpose(r(ofP[:, :]), r(outT[:, 128 * kk:128 * kk + 128]),
                                    r(idT[:, :]))
                onS = smallp.tile([128, 128], fp32, tag="onat")
                nc.scalar.copy(onS[:, :], ofP[:, :])
                nc.sync.dma_start(out=out[128 * kk:128 * kk + 128, :], in_=onS[:, :])

    nc.finalize()
    return nc


def _host_prep(item_seq, item_seq_emb, item_seq_len, W_lq, b_lq, Wq, bq, Wk, bk):
    import ml_dtypes
    bf16 = ml_dtypes.bfloat16

    emb = np.asarray(item_seq_emb, dtype=np.float32)
    seq = np.asarray(item_seq)
    slen = np.asarray(item_seq_len).astype(np.int64)

    Wqc = np.asarray(Wq, np.float32) @ np.asarray(W_lq, np.float32)
    bqc = np.asarray(Wq, np.float32) @ np.asarray(b_lq, np.float32) + np.asarray(bq, np.float32)
    Wk = np.asarray(Wk, np.float32)
    bk = np.asarray(bk, np.float32)

    j = np.arange(L)
    idx = np.clip(slen[:, None] - (j[None, :] + 1), -1, 1000)
    idx = np.where(idx < 0, idx + S, idx).astype(np.int64)
    gathered = np.take_along_axis(emb, idx[:, :, None], axis=1)     # [B,L,H]
    level_emb = np.cumsum(gathered, axis=1, dtype=np.float32)
    A = np.einsum('bli,ji->blj', level_emb, Wqc, optimize=True) + bqc  # [B,L,H]

    qW = np.empty((B, NH * L, H), np.float32)
    qb = np.empty((B, NH * L), np.float32)
    for h in range(NH):
        As = A[:, :, h * D:(h + 1) * D]
        qW[:, h * L:(h + 1) * L, :] = np.einsum('blj,ji->bli', As, Wk[h * D:(h + 1) * D, :],
                                                optimize=True)
        qb[:, h * L:(h + 1) * L] = As @ bk[h * D:(h + 1) * D]

    # qwt [cores, NGRP, H, 32*npairs]: pair block = [14 even | 14 odd | 4 zeros]
    qw6 = qW.reshape(N_CORES, NGRP, GRP // 2, 2, 14, H)
    qwt = np.zeros((N_CORES, NGRP, H, (GRP // 2) * 32), np.float32)
    qwt_v = qwt.reshape(N_CORES, NGRP, H, GRP // 2, 32)
    qwt_v[..., 0:14] = qw6[:, :, :, 0].transpose(0, 1, 4, 2, 3)
    qwt_v[..., 14:28] = qw6[:, :, :, 1].transpose(0, 1, 4, 2, 3)

    mask = (seq > 0).astype(np.float32)                              # [B,200]
    mskd = np.zeros((N_CORES, NGRP, 2, 128, 400), np.float32)
    qbd = np.zeros((N_CORES, NGRP, 128, NSG), np.float32)
    m5 = mask.reshape(N_CORES, NGRP, GRP, S)
    q5 = qb.reshape(N_CORES, NGRP, GRP, 14)
    for bl in range(GRP):
        u, r8 = bl // SG, bl % SG
        k, v = u // 4, u % 4
        q_, par = r8 // 2, r8 % 2
        r0 = 32 * v + 8 * par + 2 * q_
        for h in range(NH):
            mskd[:, :, k, r0 + h, 200 * par:200 * par + 200] = m5[:, :, bl, :]
        qbd[:, :, 32 * q_ + 14 * par:32 * q_ + 14 * par + 14, u] = 0.5 * q5[:, :, bl, :]

    selh = np.zeros((128, 32), np.float32)
    for q_ in range(4):
        for par in range(2):
            for h in range(NH):
                for l in range(L):
                    selh[32 * q_ + 14 * par + 7 * h + l, 8 * par + 2 * q_ + h] = 1.0

    emb_bf = emb.astype(bf16).reshape(N_CORES, NB, S, H)
    idnh = np.eye(128, dtype=np.float32)
    selh_bf = selh.astype(bf16)

    in_maps = []
    for c in range(N_CORES):
        in_maps.append({
            "embs": np.ascontiguousarray(emb_bf[c].transpose(1, 0, 2)),  # [S,NB,H]
            "embh": np.ascontiguousarray(emb_bf[c].transpose(2, 0, 1)),  # [H,NB,S]
            "qwt": np.ascontiguousarray(qwt[c]).astype(bf16),
            "qbp": np.ascontiguousarray(qbd[c]),
            "msk": np.ascontiguousarray(mskd[c]).astype(bf16),
            "sel": selh_bf,
            "idn": idnh,
        })
    return in_maps


def _np_fallback(item_seq, item_seq_emb, item_seq_len, W_lq, b_lq, Wq, bq, Wk, bk):
    emb = np.asarray(item_seq_emb, np.float32)
    mask = np.asarray(item_seq) > 0
    slen = np.asarray(item_seq_len).astype(np.int64)
    j = np.arange(L)
    idx = np.clip(slen[:, None] - (j[None, :] + 1), -1, 1000)
    idx = np.where(idx < 0, idx + S, idx)
    level_emb = np.cumsum(np.take_along_axis(emb, idx[:, :, None], axis=1), axis=1)
    q = ((level_emb @ np.asarray(W_lq, np.float32).T + np.asarray(b_lq, np.float32))
         @ np.asarray(Wq, np.float32).T + np.asarray(bq, np.float32)).reshape(B * NH, L, D)
    k = (emb @ np.asarray(Wk, np.float32).T + np.asarray(bk, np.float32)).reshape(B * NH, S, D)
    v = emb.reshape(B, S, NH, D)
    alpha = 1.0 / (1.0 + np.exp(-np.einsum('bld,bsd->bls', q, k, optimize=True)))
    alpha = alpha.reshape(B, NH * L, S).transpose(0, 2, 1)
    ex = np.exp(alpha - alpha.max(axis=1, keepdims=True))
    alpha = ex / ex.sum(axis=1, keepdims=True)
    alpha = np.sum(alpha.reshape(B, S, NH, L) ** 4.0, axis=-1) ** 0.25
    alpha = np.where(mask[:, :, None], alpha, -np.inf)
    ex = np.exp(alpha - alpha.max(axis=1, keepdims=True))
    alpha = ex / ex.sum(axis=1, keepdims=True)
    weighted = (alpha[..., None] * v).reshape(B, S, H) * mask[:, :, None]
    return np.sum(weighted, axis=1, dtype=np.float32).astype(np.float32)


def kernel(item_seq, item_seq_emb, item_seq_len, W_lq, b_lq, Wq, bq, Wk, bk):
    try:
        from concourse.bass_utils import run_bass_kernel_spmd

        in_maps = _host_prep(item_seq, item_seq_emb, item_seq_len,
                             W_lq, b_lq, Wq, bq, Wk, bk)
        if "nc" not in _CACHE:
            _CACHE["nc"] = _build_nc()
        res = run_bass_kernel_spmd(_CACHE["nc"], in_maps, core_ids=list(range(N_CORES)))
        _CACHE["last_result"] = res
        return np.concatenate([res.results[c]["out"] for c in range(N_CORES)], axis=0)
    except Exception as e:
        import traceback
        print(f"[kernel] device path failed ({type(e).__name__}: {e}); numpy fallback",
              flush=True)
        traceback.print_exc()
        return _np_fallback(item_seq, item_seq_emb, item_seq_len,
                            W_lq, b_lq, Wq, bq, Wk, bk)


# revision 66
# speedup vs baseline: 1.0328x; 1.0328x over previous
"""AttentionMixer Trainium2 kernel — 8-core data-parallel (batch sharded).

Host folds the projection chain (W_lq, Wq, Wk on 7 gathered rows per batch)
into per-batch query vectors qW [B,14,H] (bf16) and supplies emb in TWO
pre-transposed bf16 layouts so the device never transposes emb:
  embs [S, NB, H]  (s on partitions)  — weighted-sum stationary
  embh [H, NB, S]  (H on partitions)  — scores moving operand

Device per 64-batch group (stage-skewed software pipeline: s1 of group g+1
is hoisted a cycle early, s4 lags one group and s5 two groups, so each
in-order engine stream interleaves phases of three groups):
  s1: scores = qW @ embh (PE bf16, 4 col-tiled quadrant matmuls/supergroup),
      then sigma via 0.5*tanh(x/2)+0.5 (ACT) — tanh/exp/copy/square share one
      ACT table so the main chain is table-resident
  s2: E = exp(sigma) with Z accumulation (ACT, same table)
  s3: alpha1^2 = (E*rz^2)*E via DVE scalar_tensor_tensor; alpha1^4 via one
      GpSimd tensor_tensor square; LP pool over levels via 0/1 selection
      matmuls (PE bf16)
  s4: p = t^(1/4) (ACT sqrt x2, batched k to one table load); e0 = exp(p);
      masked softmax over S (DVE), alpha transposed via PE f32r
  s5: weighted sum via per-batch bf16 FWL stationary matmuls (PE, N=2)

Layouts (per 64-batch group, batch bl = 8*u + 2*q + par, u = 4*k + v):
  scores/sig/E tiles [128, 400] per supergroup u: row 32*q + 14*par + (7h+l),
    col 200*par + s.
  t/e tiles [128, 400] per half-group k: row 32*v + 8*par + 2*q + h.
"""

import numpy as np

N_CORES = 8
B, S, H = 2048, 200, 128
L, NH, D = 7, 2, 64
NB = B // N_CORES          # 256 batches per core
GRP = 64                   # batches per group
NGRP = NB // GRP           # 4 groups per core
SG = 8                     # batches per supergroup
NSG = GRP // SG            # 8 supergroups per group
SA, SB_ = 128, 72          # s-tile split 200 = 128 + 72

_CACHE = {}


def _build_nc():
    import concourse.bacc as bacc
    import concourse.mybir as mybir
    import concourse.tile as tile

    fp32 = mybir.dt.float32
    f32r = mybir.dt.float32r
    bf16 = mybir.dt.bfloat16
    ACT = mybir.ActivationFunctionType
    ALU = mybir.AluOpType
    AX = mybir.AxisListType

    nc = bacc.Bacc(None, target_bir_lowering=False, debug=False)

    embs = nc.declare_dram_parameter("embs", [S, NB, H], bf16, isOutput=False)
    embh = nc.declare_dram_parameter("embh", [H, NB, S], bf16, isOutput=False)
    qwt = nc.declare_dram_parameter("qwt", [NGRP, H, (GRP // 2) * 32], bf16, isOutput=False)
    qbp = nc.declare_dram_parameter("qbp", [NGRP, 128, NSG], fp32, isOutput=False)
    msk = nc.declare_dram_parameter("msk", [NGRP, 2, 128, 400], bf16, isOutput=False)
    sel = nc.declare_dram_parameter("sel", [128, 32], bf16, isOutput=False)
    idn = nc.declare_dram_parameter("idn", [128, 128], fp32, isOutput=False)
    out = nc.declare_dram_parameter("out", [NB, H], fp32, isOutput=True)

    def r(ap):
        return ap.bitcast(f32r)

    with tile.TileContext(nc) as tc:
        with (
            tc.tile_pool(name="const", bufs=1) as constp,
            tc.tile_pool(name="io", bufs=3) as iop,
            tc.tile_pool(name="embt", bufs=2) as embtp,
            tc.tile_pool(name="embs", bufs=11) as embsp,
            tc.tile_pool(name="sig", bufs=17) as sigp,
            tc.tile_pool(name="ee", bufs=8) as eep,
            tc.tile_pool(name="work", bufs=2) as workp,
            tc.tile_pool(name="e4", bufs=8) as e4p,
            tc.tile_pool(name="ens", bufs=6) as ensp,
            tc.tile_pool(name="small", bufs=2) as smallp,
            tc.tile_pool(name="psB", bufs=3, space="PSUM") as psB,
            tc.tile_pool(name="psC", bufs=1, space="PSUM") as psC,
            tc.tile_pool(name="psD", bufs=1, space="PSUM") as psD,
            tc.tile_pool(name="psE", bufs=1, space="PSUM") as psE,
        ):
            selT = constp.tile([128, 32], bf16, tag="sel")
            nc.sync.dma_start(out=selT[:, :], in_=sel[:, :])
            idT = constp.tile([128, 128], fp32, tag="idn")
            nc.sync.dma_start(out=r(idT[:, :]), in_=r(idn[:, :]))
            outT = constp.tile([128, NB], fp32, tag="outT")
            halfT = constp.tile([128, 1], fp32, tag="half")
            nc.gpsimd.memset(halfT[:, :], 0.5)

            st = {}

            def s1_scores_tanh(g):
                b0 = g * GRP
                d = st[g] = {}
                qwtT = iop.tile([H, (GRP // 2) * 32], bf16, tag="qwt")
                nc.sync.dma_start(out=qwtT[:, :], in_=qwt[g, :, :])
                mskT = [iop.tile([128, 400], bf16, tag=f"msk{k}", name=f"mskT{k}g{g}")
                        for k in range(2)]
                for k in range(2):
                    nc.sync.dma_start(out=mskT[k][:, :], in_=msk[g, k, :, :])
                qbT = iop.tile([128, NSG], fp32, tag="qb")
                nc.sync.dma_start(out=qbT[:, :], in_=qbp[g, :, :])
                embT = embtp.tile([128, GRP * S], bf16, tag="embT")
                qgrp = GRP // 4
                for qq in range(4):
                    nc.sync.dma_start(
                        out=embT[:, qq * qgrp * S:(qq + 1) * qgrp * S],
                        in_=embh[:, b0 + qq * qgrp:b0 + (qq + 1) * qgrp, :])
                embAs, embBs = [], []
                for w in range(NSG // 2):
                    bu = b0 + w * 2 * SG
                    embA = embsp.tile([SA, 2 * SG, H], bf16, tag="embA")
                    nc.sync.dma_start(out=embA[:, :, :], in_=embs[0:SA, bu:bu + 2 * SG, :])
                    embB = embsp.tile([SB_, 2 * SG, H], bf16, tag="embB")
                    nc.sync.dma_start(out=embB[:, :, :], in_=embs[SA:S, bu:bu + 2 * SG, :])
                    embAs.append(embA)
                    embBs.append(embB)
                d["mskT"], d["embAs"], d["embBs"] = mskT, embAs, embBs

                sigTs = []
                for u in range(NSG):
                    scP = psB.tile([128, 400], fp32, tag="scores")
                    for q in range(SG // 2):
                        c0 = 32 * (u * (SG // 2) + q)
                        cb = (8 * u + 2 * q) * S
                        nc.tensor.matmul(
                            scP[32 * q:32 * q + 32, :],
                            qwtT[:, c0:c0 + 32],
                            embT[:, cb:cb + 2 * S],
                            start=True, stop=True, tile_position=(0, 32 * q))
                    sigT = sigp.tile([128, 400], bf16, tag="sig")
                    nc.scalar.activation(sigT[:, :], scP[:, :], ACT.Tanh,
                                         scale=0.5, bias=qbT[:, u:u + 1])
                    sigTs.append(sigT)
                d["sigTs"] = sigTs

            def s2_exp(g):
                d = st[g]
                zT = smallp.tile([128, 2 * NSG], fp32, tag="z")
                eTs = []
                for u in range(NSG):
                    eT = eep.tile([128, 400], fp32, tag="E")
                    for par in range(2):
                        nc.scalar.activation(
                            eT[:, 200 * par:200 * par + 200],
                            d["sigTs"][u][:, 200 * par:200 * par + 200],
                            ACT.Exp, scale=0.5, bias=halfT[:, 0:1],
                            accum_out=zT[:, 2 * u + par:2 * u + par + 1])
                    eTs.append(eT)
                d["eTs"], d["zT"] = eTs, zT

            def s3_e4_lp(g):
                d = st[g]
                rzT = smallp.tile([128, 2 * NSG], fp32, tag="rz")
                nc.vector.reciprocal(rzT[:, :], d["zT"][:, :])
                rz2T = smallp.tile([128, 2 * NSG], fp32, tag="rz2")
                nc.vector.tensor_tensor(rz2T[:, :], rzT[:, :], rzT[:, :], ALU.mult)
                e4Ts = []
                for u in range(NSG):
                    # C1 = (E*rz^2)*E = (E/Z)^2 ; e4 = C1^2 = (E/Z)^4
                    cT = workp.tile([128, 400], fp32, tag="c1")
                    for par in range(2):
                        sl = slice(200 * par, 200 * par + 200)
                        nc.vector.scalar_tensor_tensor(
                            cT[:, sl], d["eTs"][u][:, sl],
                            rz2T[:, 2 * u + par:2 * u + par + 1],
                            d["eTs"][u][:, sl], ALU.mult, ALU.mult)
                    e4T = e4p.tile([128, 400], bf16, tag="e4")
                    nc.gpsimd.tensor_tensor(e4T[:, :], cT[:, :], cT[:, :], ALU.mult)
                    e4Ts.append(e4T)
                tP = [psC.tile([128, 400], fp32, tag=f"t{_k}", name=f"tP{_k}g{g}")
                      for _k in range(2)]
                for u in range(NSG):
                    k, v = u // 4, u % 4
                    nc.tensor.matmul(tP[k][32 * v:32 * v + 32, :], selT[:, :],
                                     e4Ts[u][:, :], start=True, stop=True,
                                     tile_position=(0, 32 * v))
                d["tP"] = tP

            def s4_softmax2(g):
                d = st[g]
                enSs = []
                # both k's sqrts first, then both exps: one sqrt-table load and
                # one exp-table load per group instead of two of each
                tsqs = []
                for k in range(2):
                    tsq = workp.tile([128, 400], fp32, tag="tsq")
                    nc.scalar.activation(tsq[:, :], d["tP"][k][:, :], ACT.Sqrt)
                    nc.scalar.activation(tsq[:, :], tsq[:, :], ACT.Sqrt)
                    tsqs.append(tsq)
                e0Ts = []
                for k in range(2):
                    e0T = workp.tile([128, 400], fp32, tag="e0")
                    nc.scalar.activation(e0T[:, :], tsqs[k][:, :], ACT.Exp)
                    e0Ts.append(e0T)
                for k in range(2):
                    e0T = e0Ts[k]
                    enT = workp.tile([128, 400], fp32, tag="en")
                    nc.vector.tensor_tensor(r(enT[:, :]), e0T[:, :], d["mskT"][k][:, :],
                                            ALU.mult)
                    d2T = smallp.tile([128, 2], fp32, tag="d2")
                    nc.vector.tensor_reduce(d2T[:, 0:1], enT[:, 0:200], AX.X, ALU.add)
                    nc.vector.tensor_reduce(d2T[:, 1:2], enT[:, 200:400], AX.X, ALU.add)
                    drT = smallp.tile([128, 2], fp32, tag="dr")
                    nc.vector.tensor_scalar_add(d2T[:, :], d2T[:, :], 1e-30)
                    nc.vector.reciprocal(drT[:, :], d2T[:, :])
                    for par in range(2):
                        nc.vector.tensor_scalar_mul(
                            r(enT[:, 200 * par:200 * par + 200]),
                            enT[:, 200 * par:200 * par + 200], drT[:, par:par + 1])

                    enP = psE.tile([128, 512], fp32, tag="enat")
                    nc.tensor.transpose(r(enP[:, 0:128]), r(enT[:, 0:128]), r(idT[:, :]))
                    nc.tensor.transpose(r(enP[0:SB_, 128:256]), r(enT[:, 128:200]), r(idT[:, :]))
                    nc.tensor.transpose(r(enP[:, 256:384]), r(enT[:, 200:328]), r(idT[:, :]))
                    nc.tensor.transpose(r(enP[0:SB_, 384:512]), r(enT[:, 328:400]), r(idT[:, :]))
                    enS = ensp.tile([128, 512], bf16, tag="enS")
                    nc.vector.tensor_copy(enS[:, 0:128], enP[:, 0:128])
                    nc.vector.tensor_copy(enS[0:SB_, 128:256], enP[0:SB_, 128:256])
                    nc.vector.tensor_copy(enS[:, 256:384], enP[:, 256:384])
                    nc.vector.tensor_copy(enS[0:SB_, 384:512], enP[0:SB_, 384:512])
                    enSs.append(enS)
                d["enSs"] = enSs

            def s5_weighted(g):
                d = st[g]
                embAs, embBs, enSs = d["embAs"], d["embBs"], d["enSs"]
                oaP = psD.tile([128, 2 * GRP], fp32, tag="oacc")
                for bl in range(GRP):
                    u, r8 = bl // SG, bl % SG
                    k, v = u // 4, u % 4
                    q, par = r8 // 2, r8 % 2
                    r0 = 32 * v + 8 * par + 2 * q
                    cA = 0 if par == 0 else 256
                    cB = 128 if par == 0 else 384
                    nc.tensor.matmul(
                        oaP[:, 2 * bl:2 * bl + 2],
                        embAs[u // 2][:, (u % 2) * SG + r8, :],
                        enSs[k][0:SA, cA + r0:cA + r0 + 2],
                        start=True, stop=False)
                    nc.tensor.matmul(
                        oaP[:, 2 * bl:2 * bl + 2],
                        embBs[u // 2][:, (u % 2) * SG + r8, :],
                        enSs[k][0:SB_, cB + r0:cB + r0 + 2],
                        start=False, stop=True)
                oa3 = oaP[:, :].rearrange("p (b two) -> p b two", two=2)
                nc.vector.tensor_copy(r(outT[0:64, g * GRP:(g + 1) * GRP]), oa3[0:64, :, 0])
                nc.vector.tensor_copy(r(outT[64:128, g * GRP:(g + 1) * GRP]), oa3[64:128, :, 1])
                del st[g]

            # stage-skewed emission: each engine's in-order stream interleaves
            # group g's front phases with group g-1's back phases, so no engine
            # head-of-line blocks on the e4 chain.
            s1_scores_tanh(0)
            for g in range(NGRP):
                if g + 1 < NGRP:
                    s1_scores_tanh(g + 1)
                s2_exp(g)
                s3_e4_lp(g)
                if g >= 1:
                    s4_softmax2(g - 1)
                if g >= 2:
                    s5_weighted(g - 2)
            s4_softmax2(NGRP - 1)
            s5_weighted(NGRP - 2)
            s5_weighted(NGRP - 1)

            # final transpose outT [i, b] -> out [b, i]
            for kk in range(NB // 128):
                ofP = psE.tile([128, 128], fp32, tag="enat")
                nc.tensor.transpose(r(ofP[:, :]), r(outT[:, 128 * kk:128 * kk + 128]),
                                    r(idT[:, :]))
                onS = smallp.tile([128, 128], fp32, tag="onat")
                nc.scalar.copy(onS[:, :], ofP[:, :])
                nc.sync.dma_start(out=out[128 * kk:128 * kk + 128, :], in_=onS[:, :])

    nc.finalize()
    return nc


def _host_prep(item_seq, item_seq_emb, item_seq_len, W_lq, b_lq, Wq, bq, Wk, bk):
    import ml_dtypes
    bf16 = ml_dtypes.bfloat16

    emb = np.asarray(item_seq_emb, dtype=np.float32)
    seq = np.asarray(item_seq)
    slen = np.asarray(item_seq_len).astype(np.int64)

    Wqc = np.asarray(Wq, np.float32) @ np.asarray(W_lq, np.float32)
    bqc = np.asarray(Wq, np.float32) @ np.asarray(b_lq, np.float32) + np.asarray(bq, np.float32)
    Wk = np.asarray(Wk, np.float32)
    bk = np.asarray(bk, np.float32)

    j = np.arange(L)
    idx = np.clip(slen[:, None] - (j[None, :] + 1), -1, 1000)
    idx = np.where(idx < 0, idx + S, idx).astype(np.int64)
    gathered = np.take_along_axis(emb, idx[:, :, None], axis=1)     # [B,L,H]
    level_emb = np.cumsum(gathered, axis=1, dtype=np.float32)
    A = np.einsum('bli,ji->blj', level_emb, Wqc, optimize=True) + bqc  # [B,L,H]

    qW = np.empty((B, NH * L, H), np.float32)
    qb = np.empty((B, NH * L), np.float32)
    for h in range(NH):
        As = A[:, :, h * D:(h + 1) * D]
        qW[:, h * L:(h + 1) * L, :] = np.einsum('blj,ji->bli', As, Wk[h * D:(h + 1) * D, :],
                                                optimize=True)
        qb[:, h * L:(h + 1) * L] = As @ bk[h * D:(h + 1) * D]

    # qwt [cores, NGRP, H, 32*npairs]: pair block = [14 even | 14 odd | 4 zeros]
    qw6 = qW.reshape(N_CORES, NGRP, GRP // 2, 2, 14, H)
    qwt = np.zeros((N_CORES, NGRP, H, (GRP // 2) * 32), np.float32)
    qwt_v = qwt.reshape(N_CORES, NGRP, H, GRP // 2, 32)
    qwt_v[..., 0:14] = qw6[:, :, :, 0].transpose(0, 1, 4, 2, 3)
    qwt_v[..., 14:28] = qw6[:, :, :, 1].transpose(0, 1, 4, 2, 3)

    mask = (seq > 0).astype(np.float32)                              # [B,200]
    mskd = np.zeros((N_CORES, NGRP, 2, 128, 400), np.float32)
    qbd = np.zeros((N_CORES, NGRP, 128, NSG), np.float32)
    m5 = mask.reshape(N_CORES, NGRP, GRP, S)
    q5 = qb.reshape(N_CORES, NGRP, GRP, 14)
    for bl in range(GRP):
        u, r8 = bl // SG, bl % SG
        k, v = u // 4, u % 4
        q_, par = r8 // 2, r8 % 2
        r0 = 32 * v + 8 * par + 2 * q_
        for h in range(NH):
            mskd[:, :, k, r0 + h, 200 * par:200 * par + 200] = m5[:, :, bl, :]
        qbd[:, :, 32 * q_ + 14 * par:32 * q_ + 14 * par + 14, u] = 0.5 * q5[:, :, bl, :]

    selh = np.zeros((128, 32), np.float32)
    for q_ in range(4):
        for par in range(2):
            for h in range(NH):
                for l in range(L):
                    selh[32 * q_ + 14 * par + 7 * h + l, 8 * par + 2 * q_ + h] = 1.0

    emb_bf = emb.astype(bf16).reshape(N_CORES, NB, S, H)
    idnh = np.eye(128, dtype=np.float32)
    selh_bf = selh.astype(bf16)

    in_maps = []
    for c in range(N_CORES):
        in_maps.append({
            "embs": np.ascontiguousarray(emb_bf[c].transpose(1, 0, 2)),  # [S,NB,H]
            "embh": np.ascontiguousarray(emb_bf[c].transpose(2, 0, 1)),  # [H,NB,S]
            "qwt": np.ascontiguousarray(qwt[c]).astype(bf16),
            "qbp": np.ascontiguousarray(qbd[c]),
            "msk": np.ascontiguousarray(mskd[c]).astype(bf16),
            "sel": selh_bf,
            "idn": idnh,
        })
    return in_maps


def _np_fallback(item_seq, item_seq_emb, item_seq_len, W_lq, b_lq, Wq, bq, Wk, bk):
    emb = np.asarray(item_seq_emb, np.float32)
    mask = np.asarray(item_seq) > 0
    slen = np.asarray(item_seq_len).astype(np.int64)
    j = np.arange(L)
    idx = np.clip(slen[:, None] - (j[None, :] + 1), -1, 1000)
    idx = np.where(idx < 0, idx + S, idx)
    level_emb = np.cumsum(np.take_along_axis(emb, idx[:, :, None], axis=1), axis=1)
    q = ((level_emb @ np.asarray(W_lq, np.float32).T + np.asarray(b_lq, np.float32))
         @ np.asarray(Wq, np.float32).T + np.asarray(bq, np.float32)).reshape(B * NH, L, D)
    k = (emb @ np.asarray(Wk, np.float32).T + np.asarray(bk, np.float32)).reshape(B * NH, S, D)
    v = emb.reshape(B, S, NH, D)
    alpha = 1.0 / (1.0 + np.exp(-np.einsum('bld,bsd->bls', q, k, optimize=True)))
    alpha = alpha.reshape(B, NH * L, S).transpose(0, 2, 1)
    ex = np.exp(alpha - alpha.max(axis=1, keepdims=True))
    alpha = ex / ex.sum(axis=1, keepdims=True)
    alpha = np.sum(alpha.reshape(B, S, NH, L) ** 4.0, axis=-1) ** 0.25
    alpha = np.where(mask[:, :, None], alpha, -np.inf)
    ex = np.exp(alpha - alpha.max(axis=1, keepdims=True))
    alpha = ex / ex.sum(axis=1, keepdims=True)
    weighted = (alpha[..., None] * v).reshape(B, S, H) * mask[:, :, None]
    return np.sum(weighted, axis=1, dtype=np.float32).astype(np.float32)


def kernel(item_seq, item_seq_emb, item_seq_len, W_lq, b_lq, Wq, bq, Wk, bk):
    try:
        from concourse.bass_utils import run_bass_kernel_spmd

        in_maps = _host_prep(item_seq, item_seq_emb, item_seq_len,
                             W_lq, b_lq, Wq, bq, Wk, bk)
        if "nc" not in _CACHE:
            _CACHE["nc"] = _build_nc()
        res = run_bass_kernel_spmd(_CACHE["nc"], in_maps, core_ids=list(range(N_CORES)))
        _CACHE["last_result"] = res
        return np.concatenate([res.results[c]["out"] for c in range(N_CORES)], axis=0)
    except Exception as e:
        import traceback
        print(f"[kernel] device path failed ({type(e).__name__}: {e}); numpy fallback",
              flush=True)
        traceback.print_exc()
        return _np_fallback(item_seq, item_seq_emb, item_seq_len,
                            W_lq, b_lq, Wq, bq, Wk, bk)


# revision 67
# speedup vs baseline: 1.0347x; 1.0018x over previous
"""AttentionMixer Trainium2 kernel — 8-core data-parallel (batch sharded).

Host folds the projection chain (W_lq, Wq, Wk on 7 gathered rows per batch)
into per-batch query vectors qW [B,14,H] (bf16) and supplies emb in TWO
pre-transposed bf16 layouts so the device never transposes emb:
  embs [S, NB, H]  (s on partitions)  — weighted-sum stationary
  embh [H, NB, S]  (H on partitions)  — scores moving operand

Device per 64-batch group (stage-skewed software pipeline: s1 of group g+1
is hoisted a cycle early, s4 lags one group and s5 two groups, so each
in-order engine stream interleaves phases of three groups):
  s1: scores = qW @ embh (PE bf16, 4 col-tiled quadrant matmuls/supergroup),
      then sigma via 0.5*tanh(x/2)+0.5 (ACT) — tanh/exp/copy/square share one
      ACT table so the main chain is table-resident
  s2: E = exp(sigma) with Z accumulation (ACT, same table)
  s3: alpha1^2 = (E*rz^2)*E via DVE scalar_tensor_tensor; alpha1^4 via one
      GpSimd tensor_tensor square; LP pool over levels via 0/1 selection
      matmuls (PE bf16)
  s4: p = t^(1/4) (ACT sqrt x2, batched k to one table load); e0 = exp(p);
      masked softmax over S (DVE), alpha transposed via PE f32r
  s5: weighted sum via per-batch bf16 FWL stationary matmuls (PE, N=2)

Layouts (per 64-batch group, batch bl = 8*u + 2*q + par, u = 4*k + v):
  scores/sig/E tiles [128, 400] per supergroup u: row 32*q + 14*par + (7h+l),
    col 200*par + s.
  t/e tiles [128, 400] per half-group k: row 32*v + 8*par + 2*q + h.
"""

import numpy as np

N_CORES = 8
B, S, H = 2048, 200, 128
L, NH, D = 7, 2, 64
NB = B // N_CORES          # 256 batches per core
GRP = 64                   # batches per group
NGRP = NB // GRP           # 4 groups per core
SG = 8                     # batches per supergroup
NSG = GRP // SG            # 8 supergroups per group
SA, SB_ = 128, 72          # s-tile split 200 = 128 + 72

_CACHE = {}


def _build_nc():
    import concourse.bacc as bacc
    import concourse.mybir as mybir
    import concourse.tile as tile

    fp32 = mybir.dt.float32
    f32r = mybir.dt.float32r
    bf16 = mybir.dt.bfloat16
    ACT = mybir.ActivationFunctionType
    ALU = mybir.AluOpType
    AX = mybir.AxisListType

    nc = bacc.Bacc(None, target_bir_lowering=False, debug=False)

    embs = nc.declare_dram_parameter("embs", [S, NB, H], bf16, isOutput=False)
    embh = nc.declare_dram_parameter("embh", [H, NB, S], bf16, isOutput=False)
    qwt = nc.declare_dram_parameter("qwt", [NGRP, H, (GRP // 2) * 32], bf16, isOutput=False)
    qbp = nc.declare_dram_parameter("qbp", [NGRP, 128, NSG], fp32, isOutput=False)
    msk = nc.declare_dram_parameter("msk", [NGRP, 2, 128, 400], bf16, isOutput=False)
    sel = nc.declare_dram_parameter("sel", [128, 32], bf16, isOutput=False)
    idn = nc.declare_dram_parameter("idn", [128, 128], fp32, isOutput=False)
    out = nc.declare_dram_parameter("out", [NB, H], fp32, isOutput=True)

    def r(ap):
        return ap.bitcast(f32r)

    with tile.TileContext(nc) as tc:
        with (
            tc.tile_pool(name="const", bufs=1) as constp,
            tc.tile_pool(name="io", bufs=3) as iop,
            tc.tile_pool(name="embt", bufs=2) as embtp,
            tc.tile_pool(name="embs", bufs=11) as embsp,
            tc.tile_pool(name="sig", bufs=17) as sigp,
            tc.tile_pool(name="ee", bufs=8) as eep,
            tc.tile_pool(name="work", bufs=2) as workp,
            tc.tile_pool(name="e4", bufs=8) as e4p,
            tc.tile_pool(name="ens", bufs=6) as ensp,
            tc.tile_pool(name="small", bufs=2) as smallp,
            tc.tile_pool(name="psB", bufs=3, space="PSUM") as psB,
            tc.tile_pool(name="psC", bufs=1, space="PSUM") as psC,
            tc.tile_pool(name="psD", bufs=1, space="PSUM") as psD,
            tc.tile_pool(name="psE", bufs=1, space="PSUM") as psE,
        ):
            selT = constp.tile([128, 32], bf16, tag="sel")
            nc.sync.dma_start(out=selT[:, :], in_=sel[:, :])
            idT = constp.tile([128, 128], fp32, tag="idn")
            nc.sync.dma_start(out=r(idT[:, :]), in_=r(idn[:, :]))
            outT = constp.tile([128, NB], fp32, tag="outT")
            halfT = constp.tile([128, 1], fp32, tag="half")
            nc.gpsimd.memset(halfT[:, :], 0.5)

            st = {}

            def s1_scores_tanh(g):
                b0 = g * GRP
                d = st[g] = {}
                embT = embtp.tile([128, GRP * S], bf16, tag="embT")
                qwtT = iop.tile([H, (GRP // 2) * 32], bf16, tag="qwt")
                qgrp = GRP // 4
                nc.sync.dma_start(out=embT[:, 0:qgrp * S],
                                  in_=embh[:, b0:b0 + qgrp, :])
                nc.sync.dma_start(out=qwtT[:, :], in_=qwt[g, :, :])
                for qq in range(1, 4):
                    nc.sync.dma_start(
                        out=embT[:, qq * qgrp * S:(qq + 1) * qgrp * S],
                        in_=embh[:, b0 + qq * qgrp:b0 + (qq + 1) * qgrp, :])
                mskT = [iop.tile([128, 400], bf16, tag=f"msk{k}", name=f"mskT{k}g{g}")
                        for k in range(2)]
                for k in range(2):
                    nc.sync.dma_start(out=mskT[k][:, :], in_=msk[g, k, :, :])
                qbT = iop.tile([128, NSG], fp32, tag="qb")
                nc.sync.dma_start(out=qbT[:, :], in_=qbp[g, :, :])
                embAs, embBs = [], []
                for w in range(NSG // 2):
                    bu = b0 + w * 2 * SG
                    embA = embsp.tile([SA, 2 * SG, H], bf16, tag="embA")
                    nc.sync.dma_start(out=embA[:, :, :], in_=embs[0:SA, bu:bu + 2 * SG, :])
                    embB = embsp.tile([SB_, 2 * SG, H], bf16, tag="embB")
                    nc.sync.dma_start(out=embB[:, :, :], in_=embs[SA:S, bu:bu + 2 * SG, :])
                    embAs.append(embA)
                    embBs.append(embB)
                d["mskT"], d["embAs"], d["embBs"] = mskT, embAs, embBs

                sigTs = []
                for u in range(NSG):
                    scP = psB.tile([128, 400], fp32, tag="scores")
                    for q in range(SG // 2):
                        c0 = 32 * (u * (SG // 2) + q)
                        cb = (8 * u + 2 * q) * S
                        nc.tensor.matmul(
                            scP[32 * q:32 * q + 32, :],
                            qwtT[:, c0:c0 + 32],
                            embT[:, cb:cb + 2 * S],
                            start=True, stop=True, tile_position=(0, 32 * q))
                    sigT = sigp.tile([128, 400], bf16, tag="sig")
                    nc.scalar.activation(sigT[:, :], scP[:, :], ACT.Tanh,
                                         scale=0.5, bias=qbT[:, u:u + 1])
                    sigTs.append(sigT)
                d["sigTs"] = sigTs

            def s2_exp(g):
                d = st[g]
                zT = smallp.tile([128, 2 * NSG], fp32, tag="z")
                eTs = []
                for u in range(NSG):
                    eT = eep.tile([128, 400], fp32, tag="E")
                    for par in range(2):
                        nc.scalar.activation(
                            eT[:, 200 * par:200 * par + 200],
                            d["sigTs"][u][:, 200 * par:200 * par + 200],
                            ACT.Exp, scale=0.5, bias=halfT[:, 0:1],
                            accum_out=zT[:, 2 * u + par:2 * u + par + 1])
                    eTs.append(eT)
                d["eTs"], d["zT"] = eTs, zT

            def s3_e4_lp(g):
                d = st[g]
                rzT = smallp.tile([128, 2 * NSG], fp32, tag="rz")
                nc.vector.reciprocal(rzT[:, :], d["zT"][:, :])
                rz2T = smallp.tile([128, 2 * NSG], fp32, tag="rz2")
                nc.vector.tensor_tensor(rz2T[:, :], rzT[:, :], rzT[:, :], ALU.mult)
                e4Ts = []
                for u in range(NSG):
                    # C1 = (E*rz^2)*E = (E/Z)^2 ; e4 = C1^2 = (E/Z)^4
                    cT = workp.tile([128, 400], fp32, tag="c1")
                    for par in range(2):
                        sl = slice(200 * par, 200 * par + 200)
                        nc.vector.scalar_tensor_tensor(
                            cT[:, sl], d["eTs"][u][:, sl],
                            rz2T[:, 2 * u + par:2 * u + par + 1],
                            d["eTs"][u][:, sl], ALU.mult, ALU.mult)
                    e4T = e4p.tile([128, 400], bf16, tag="e4")
                    nc.gpsimd.tensor_tensor(e4T[:, :], cT[:, :], cT[:, :], ALU.mult)
                    e4Ts.append(e4T)
                tP = [psC.tile([128, 400], fp32, tag=f"t{_k}", name=f"tP{_k}g{g}")
                      for _k in range(2)]
                for u in range(NSG):
                    k, v = u // 4, u % 4
                    nc.tensor.matmul(tP[k][32 * v:32 * v + 32, :], selT[:, :],
                                     e4Ts[u][:, :], start=True, stop=True,
                                     tile_position=(0, 32 * v))
                d["tP"] = tP

            def s4_softmax2(g):
                d = st[g]
                enSs = []
                # both k's sqrts first, then both exps: one sqrt-table load and
                # one exp-table load per group instead of two of each
                tsqs = []
                for k in range(2):
                    tsq = workp.tile([128, 400], fp32, tag="tsq")
                    nc.scalar.activation(tsq[:, :], d["tP"][k][:, :], ACT.Sqrt)
                    nc.scalar.activation(tsq[:, :], tsq[:, :], ACT.Sqrt)
                    tsqs.append(tsq)
                e0Ts = []
                for k in range(2):
                    e0T = workp.tile([128, 400], fp32, tag="e0")
                    nc.scalar.activation(e0T[:, :], tsqs[k][:, :], ACT.Exp)
                    e0Ts.append(e0T)
                for k in range(2):
                    e0T = e0Ts[k]
                    enT = workp.tile([128, 400], fp32, tag="en")
                    nc.vector.tensor_tensor(r(enT[:, :]), e0T[:, :], d["mskT"][k][:, :],
                                            ALU.mult)
                    d2T = smallp.tile([128, 2], fp32, tag="d2")
                    nc.vector.tensor_reduce(d2T[:, 0:1], enT[:, 0:200], AX.X, ALU.add)
                    nc.vector.tensor_reduce(d2T[:, 1:2], enT[:, 200:400], AX.X, ALU.add)
                    drT = smallp.tile([128, 2], fp32, tag="dr")
                    nc.vector.tensor_scalar_add(d2T[:, :], d2T[:, :], 1e-30)
                    nc.vector.reciprocal(drT[:, :], d2T[:, :])
                    for par in range(2):
                        nc.vector.tensor_scalar_mul(
                            r(enT[:, 200 * par:200 * par + 200]),
                            enT[:, 200 * par:200 * par + 200], drT[:, par:par + 1])

                    enP = psE.tile([128, 512], fp32, tag="enat")
                    nc.tensor.transpose(r(enP[:, 0:128]), r(enT[:, 0:128]), r(idT[:, :]))
                    nc.tensor.transpose(r(enP[0:SB_, 128:256]), r(enT[:, 128:200]), r(idT[:, :]))
                    nc.tensor.transpose(r(enP[:, 256:384]), r(enT[:, 200:328]), r(idT[:, :]))
                    nc.tensor.transpose(r(enP[0:SB_, 384:512]), r(enT[:, 328:400]), r(idT[:, :]))
                    enS = ensp.tile([128, 512], bf16, tag="enS")
                    nc.vector.tensor_copy(enS[:, 0:128], enP[:, 0:128])
                    nc.vector.tensor_copy(enS[0:SB_, 128:256], enP[0:SB_, 128:256])
                    nc.vector.tensor_copy(enS[:, 256:384], enP[:, 256:384])
                    nc.vector.tensor_copy(enS[0:SB_, 384:512], enP[0:SB_, 384:512])
                    enSs.append(enS)
                d["enSs"] = enSs

            def s5_weighted(g):
                d = st[g]
                embAs, embBs, enSs = d["embAs"], d["embBs"], d["enSs"]
                oaP = psD.tile([128, 2 * GRP], fp32, tag="oacc")
                for bl in range(GRP):
                    u, r8 = bl // SG, bl % SG
                    k, v = u // 4, u % 4
                    q, par = r8 // 2, r8 % 2
                    r0 = 32 * v + 8 * par + 2 * q
                    cA = 0 if par == 0 else 256
                    cB = 128 if par == 0 else 384
                    nc.tensor.matmul(
                        oaP[:, 2 * bl:2 * bl + 2],
                        embAs[u // 2][:, (u % 2) * SG + r8, :],
                        enSs[k][0:SA, cA + r0:cA + r0 + 2],
                        start=True, stop=False)
                    nc.tensor.matmul(
                        oaP[:, 2 * bl:2 * bl + 2],
                        embBs[u // 2][:, (u % 2) * SG + r8, :],
                        enSs[k][0:SB_, cB + r0:cB + r0 + 2],
                        start=False, stop=True)
                oa3 = oaP[:, :].rearrange("p (b two) -> p b two", two=2)
                nc.vector.tensor_copy(r(outT[0:64, g * GRP:(g + 1) * GRP]), oa3[0:64, :, 0])
                nc.vector.tensor_copy(r(outT[64:128, g * GRP:(g + 1) * GRP]), oa3[64:128, :, 1])
                del st[g]

            # stage-skewed emission: each engine's in-order stream interleaves
            # group g's front phases with group g-1's back phases, so no engine
            # head-of-line blocks on the e4 chain.
            s1_scores_tanh(0)
            for g in range(NGRP):
                if g + 1 < NGRP:
                    s1_scores_tanh(g + 1)
                s2_exp(g)
                s3_e4_lp(g)
                if g >= 1:
                    s4_softmax2(g - 1)
                if g >= 2:
                    s5_weighted(g - 2)
            s4_softmax2(NGRP - 1)
            s5_weighted(NGRP - 2)
            s5_weighted(NGRP - 1)

            # final transpose outT [i, b] -> out [b, i]
            for kk in range(NB // 128):
                ofP = psE.tile([128, 128], fp32, tag="enat")
                nc.tensor.transpose(r(ofP[:, :]), r(outT[:, 128 * kk:128 * kk + 128]),
                                    r(idT[:, :]))
                onS = smallp.tile([128, 128], fp32, tag="onat")
                nc.scalar.copy(onS[:, :], ofP[:, :])
                nc.sync.dma_start(out=out[128 * kk:128 * kk + 128, :], in_=onS[:, :])

    nc.finalize()
    return nc


def _host_prep(item_seq, item_seq_emb, item_seq_len, W_lq, b_lq, Wq, bq, Wk, bk):
    import ml_dtypes
    bf16 = ml_dtypes.bfloat16

    emb = np.asarray(item_seq_emb, dtype=np.float32)
    seq = np.asarray(item_seq)
    slen = np.asarray(item_seq_len).astype(np.int64)

    Wqc = np.asarray(Wq, np.float32) @ np.asarray(W_lq, np.float32)
    bqc = np.asarray(Wq, np.float32) @ np.asarray(b_lq, np.float32) + np.asarray(bq, np.float32)
    Wk = np.asarray(Wk, np.float32)
    bk = np.asarray(bk, np.float32)

    j = np.arange(L)
    idx = np.clip(slen[:, None] - (j[None, :] + 1), -1, 1000)
    idx = np.where(idx < 0, idx + S, idx).astype(np.int64)
    gathered = np.take_along_axis(emb, idx[:, :, None], axis=1)     # [B,L,H]
    level_emb = np.cumsum(gathered, axis=1, dtype=np.float32)
    A = np.einsum('bli,ji->blj', level_emb, Wqc, optimize=True) + bqc  # [B,L,H]

    qW = np.empty((B, NH * L, H), np.float32)
    qb = np.empty((B, NH * L), np.float32)
    for h in range(NH):
        As = A[:, :, h * D:(h + 1) * D]
        qW[:, h * L:(h + 1) * L, :] = np.einsum('blj,ji->bli', As, Wk[h * D:(h + 1) * D, :],
                                                optimize=True)
        qb[:, h * L:(h + 1) * L] = As @ bk[h * D:(h + 1) * D]

    # qwt [cores, NGRP, H, 32*npairs]: pair block = [14 even | 14 odd | 4 zeros]
    qw6 = qW.reshape(N_CORES, NGRP, GRP // 2, 2, 14, H)
    qwt = np.zeros((N_CORES, NGRP, H, (GRP // 2) * 32), np.float32)
    qwt_v = qwt.reshape(N_CORES, NGRP, H, GRP // 2, 32)
    qwt_v[..., 0:14] = qw6[:, :, :, 0].transpose(0, 1, 4, 2, 3)
    qwt_v[..., 14:28] = qw6[:, :, :, 1].transpose(0, 1, 4, 2, 3)

    mask = (seq > 0).astype(np.float32)                              # [B,200]
    mskd = np.zeros((N_CORES, NGRP, 2, 128, 400), np.float32)
    qbd = np.zeros((N_CORES, NGRP, 128, NSG), np.float32)
    m5 = mask.reshape(N_CORES, NGRP, GRP, S)
    q5 = qb.reshape(N_CORES, NGRP, GRP, 14)
    for bl in range(GRP):
        u, r8 = bl // SG, bl % SG
        k, v = u // 4, u % 4
        q_, par = r8 // 2, r8 % 2
        r0 = 32 * v + 8 * par + 2 * q_
        for h in range(NH):
            mskd[:, :, k, r0 + h, 200 * par:200 * par + 200] = m5[:, :, bl, :]
        qbd[:, :, 32 * q_ + 14 * par:32 * q_ + 14 * par + 14, u] = 0.5 * q5[:, :, bl, :]

    selh = np.zeros((128, 32), np.float32)
    for q_ in range(4):
        for par in range(2):
            for h in range(NH):
                for l in range(L):
                    selh[32 * q_ + 14 * par + 7 * h + l, 8 * par + 2 * q_ + h] = 1.0

    emb_bf = emb.astype(bf16).reshape(N_CORES, NB, S, H)
    idnh = np.eye(128, dtype=np.float32)
    selh_bf = selh.astype(bf16)

    in_maps = []
    for c in range(N_CORES):
        in_maps.append({
            "embs": np.ascontiguousarray(emb_bf[c].transpose(1, 0, 2)),  # [S,NB,H]
            "embh": np.ascontiguousarray(emb_bf[c].transpose(2, 0, 1)),  # [H,NB,S]
            "qwt": np.ascontiguousarray(qwt[c]).astype(bf16),
            "qbp": np.ascontiguousarray(qbd[c]),
            "msk": np.ascontiguousarray(mskd[c]).astype(bf16),
            "sel": selh_bf,
            "idn": idnh,
        })
    return in_maps


def _np_fallback(item_seq, item_seq_emb, item_seq_len, W_lq, b_lq, Wq, bq, Wk, bk):
    emb = np.asarray(item_seq_emb, np.float32)
    mask = np.asarray(item_seq) > 0
    slen = np.asarray(item_seq_len).astype(np.int64)
    j = np.arange(L)
    idx = np.clip(slen[:, None] - (j[None, :] + 1), -1, 1000)
    idx = np.where(idx < 0, idx + S, idx)
    level_emb = np.cumsum(np.take_along_axis(emb, idx[:, :, None], axis=1), axis=1)
    q = ((level_emb @ np.asarray(W_lq, np.float32).T + np.asarray(b_lq, np.float32))
         @ np.asarray(Wq, np.float32).T + np.asarray(bq, np.float32)).reshape(B * NH, L, D)
    k = (emb @ np.asarray(Wk, np.float32).T + np.asarray(bk, np.float32)).reshape(B * NH, S, D)
    v = emb.reshape(B, S, NH, D)
    alpha = 1.0 / (1.0 + np.exp(-np.einsum('bld,bsd->bls', q, k, optimize=True)))
    alpha = alpha.reshape(B, NH * L, S).transpose(0, 2, 1)
    ex = np.exp(alpha - alpha.max(axis=1, keepdims=True))
    alpha = ex / ex.sum(axis=1, keepdims=True)
    alpha = np.sum(alpha.reshape(B, S, NH, L) ** 4.0, axis=-1) ** 0.25
    alpha = np.where(mask[:, :, None], alpha, -np.inf)
    ex = np.exp(alpha - alpha.max(axis=1, keepdims=True))
    alpha = ex / ex.sum(axis=1, keepdims=True)
    weighted = (alpha[..., None] * v).reshape(B, S, H) * mask[:, :, None]
    return np.sum(weighted, axis=1, dtype=np.float32).astype(np.float32)


def kernel(item_seq, item_seq_emb, item_seq_len, W_lq, b_lq, Wq, bq, Wk, bk):
    try:
        from concourse.bass_utils import run_bass_kernel_spmd

        in_maps = _host_prep(item_seq, item_seq_emb, item_seq_len,
                             W_lq, b_lq, Wq, bq, Wk, bk)
        if "nc" not in _CACHE:
            _CACHE["nc"] = _build_nc()
        res = run_bass_kernel_spmd(_CACHE["nc"], in_maps, core_ids=list(range(N_CORES)))
        _CACHE["last_result"] = res
        return np.concatenate([res.results[c]["out"] for c in range(N_CORES)], axis=0)
    except Exception as e:
        import traceback
        print(f"[kernel] device path failed ({type(e).__name__}: {e}); numpy fallback",
              flush=True)
        traceback.print_exc()
        return _np_fallback(item_seq, item_seq_emb, item_seq_len,
                            W_lq, b_lq, Wq, bq, Wk, bk)
